# revision 16
# baseline (speedup 1.0000x reference)
"""Trainium2 Bass kernel for nn_ObjectRaysampler.

Full (unsharded) inputs -> full outputs. Rays are sharded across 8 NeuronCores
(data-parallel); the tiny object transforms are replicated.

Per super-tile of G*128 rays (128 partitions x G ray-groups in the free dim):
  - world->object transform of ray origins/directions (row-vector convention),
    slab ray-AABB test against the unit box, entry/exit t.
  - world-space sample depths are z_w = c * z_o with c = ||d|| / ||d @ (R S)||,
    so each object's 32 samples form an ascending arithmetic progression:
    z(s) = A + B*(s/31), A = c*t_in, B = c*(tmax-t_in); misses get exactly 1e10.
  - sort keys: the f32 depth with its low 5 mantissa bits replaced by
    (node_id+1) for hits and 0 for misses/base entries. Keys stay positive
    floats, so min/max comparisons reproduce the reference's stable argsort
    order (remaining ties are between interchangeable entries), and node id /
    hit mask are recovered from the sorted keys by bit masking. The <=31-ulp
    key perturbation is ~4e-6 relative on the output lengths.
  - the per-ray 576 values are 17 presorted runs (16 arithmetic progressions
    + the presorted base lengths), merged with a bitonic-merge network of
    40 stages / ~96 min-max ops on 576-slot ping-pong buffers (the inf-pad
    of the final merge provably collapses to 64-wide ops). Early phases run
    on GpSimd, later phases on VectorE, so consecutive super-tiles pipeline
    across both engines.
  - sample points/dirs are affine in s: pts = P + Q*s per (ray, object, xyz),
    so TensorE produces them as [P|Q] @ E against a constant selector matrix,
    ScalarE drains PSUM->SBUF, and DMA streams them to HBM.
"""

import contextlib
import numpy as np

from concourse import bacc, tile, mybir
from concourse.alu_op_type import AluOpType
from concourse.bass_utils import run_bass_kernel_spmd

N_RAYS = 32768
M = 16          # objects
S = 32          # samples per object
B = 64          # base samples
K = B + M * S   # 576
MISS = 1e10
N_CORES = 8
P = 128                        # partition dim (rays per group)
G = 4                          # ray-groups per super-tile
CORE_RAYS = N_RAYS // N_CORES  # 4096
F32 = mybir.dt.float32
I32 = mybir.dt.int32
U8 = mybir.dt.uint8

# ---------------------------------------------------------------- sort network


def _flip(L, lo, hi):
    nblk = (hi - lo) // (2 * L)
    q = 2 * L
    return [
        ("min", lo, nblk, q, slice(0, L), slice(0, L), slice(2 * L - 1, L - 1, -1)),
        ("max", lo, nblk, q, slice(L, 2 * L), slice(L, 2 * L), slice(L - 1, None, -1)),
    ]


def _plain(d, lo, hi):
    nblk = (hi - lo) // (2 * d)
    q = 2 * d
    return [
        ("min", lo, nblk, q, slice(0, d), slice(0, d), slice(d, 2 * d)),
        ("max", lo, nblk, q, slice(d, 2 * d), slice(0, d), slice(d, 2 * d)),
    ]


N_POOL_SORT = 0   # GpSimd sort assist disabled: POOL ALU is float-only,
                  # so exact int32 compare-exchange is impossible there


def build_stages2():
    stages = []
    for L in (32, 64, 128, 256):            # merge 32-runs -> one 512-run in [0,512)
        stages.append(_flip(L, 0, 512))
        d = L // 2
        while d >= 1:
            stages.append(_plain(d, 0, 512))
            d //= 2
    return stages


def build_final_merge():
    """Returns (lower_stages, upper_stages) for the compact final merge.

    The virtual-inf-pad flip of the 1024-wide bitonic merge collapses to the
    64 real pairs and runs IN-PLACE (max first, reading only originals; min
    then reads the stashed original base from a temp). lower_stages sort the
    bitonic [0,512); upper_stages sort the bitonic [512,576) independently.
    """
    lower = [_plain(d, 0, 512) for d in (256, 128, 64, 32, 16, 8, 4, 2, 1)]
    upper = [_plain(d, 512, 576) for d in (32, 16, 8, 4, 2, 1)]
    return lower, upper


_ALU = {"min": AluOpType.min, "max": AluOpType.max}


def _spec_ap(buf, g, spec):
    """buf: [P, G*K] AP; spec = (off, step, cnt) within each group's K slots."""
    off, step, cnt = spec
    v = buf.rearrange("p (g k) -> p g k", g=g)
    if step > 0:
        return v[:, :, off:off + cnt]
    stop = off - cnt
    return v[:, :, off:stop if stop >= 0 else None:-1]


def _emit_stage(nc, eng, dst, src, stage, g):
    e = nc.vector if eng == "v" else nc.gpsimd
    for op in stage:
        kind = op[0]
        if isinstance(op[1], tuple):
            _, o, a, b = op
            dv = _spec_ap(dst, g, o)
            av = _spec_ap(src, g, a)
            bv = _spec_ap(src, g, b) if b is not None else None
        else:
            _, off, nblk, q, o_sl, a_sl, b_sl = op
            dvv = dst.rearrange("p (g k) -> p g k", g=g)[:, :, off:off + nblk * q]
            avv = src.rearrange("p (g k) -> p g k", g=g)[:, :, off:off + nblk * q]
            dv = dvv.rearrange("p g (b q) -> p g b q", b=nblk)[:, :, :, o_sl]
            av = avv.rearrange("p g (b q) -> p g b q", b=nblk)[:, :, :, a_sl]
            bv = avv.rearrange("p g (b q) -> p g b q", b=nblk)[:, :, :, b_sl]
        if kind == "copy":
            e.tensor_copy(dv, av)
        else:
            e.tensor_tensor(dv, av, bv, _ALU[kind])


def _pair_views(dst, src, g, op_min):
    """From a 'min' op tuple, build (in0, in1, out_min, out_max_in_fwd, r_rev?)
    views: in0/in1 are the comparator sides; outputs at in0/in1 positions."""
    if isinstance(op_min[1], tuple):
        _, o, a, b = op_min
        raise NotImplementedError
    _, off, nblk, q, o_sl, a_sl, b_sl = op_min
    dvv = dst.rearrange("p (g k) -> p g k", g=g)[:, :, off:off + nblk * q]
    svv = src.rearrange("p (g k) -> p g k", g=g)[:, :, off:off + nblk * q]
    din = svv.rearrange("p g (b q) -> p g b q", b=nblk)
    dout = dvv.rearrange("p g (b q) -> p g b q", b=nblk)
    return din, dout, a_sl, b_sl


def _emit_pool_stage(nc, dst, src, stage, g, scratch):
    """Exact compare-exchange on GpSimd via int32 arithmetic:
    r = relu(a - b); min = a - r at a-positions; max = b + r at b-positions.
    Works because positive-f32 key order == int32 order and int ops are exact.
    `stage` must be a [min, max] pair from _flip/_plain."""
    I32 = mybir.dt.int32
    op_min, op_max = stage
    assert op_min[0] == "min" and op_max[0] == "max"
    _, off, nblk, q, o_sl, a_sl, b_sl = op_min
    din, dout, _, _ = _pair_views(dst[:].bitcast(I32), src[:].bitcast(I32), g, op_min)
    a = din[:, :, :, a_sl]
    b = din[:, :, :, b_sl]
    w = a.shape[-1]
    sc = scratch[:, 0:nblk * g * w * 0 + g * nblk * w].rearrange(
        "p (g b w) -> p g b w", g=g, b=nblk)
    nc.gpsimd.tensor_tensor(sc, a, b, AluOpType.subtract)
    nc.gpsimd.tensor_relu(sc, sc)
    nc.gpsimd.tensor_tensor(dout[:, :, :, a_sl], a, sc, AluOpType.subtract)
    # max at b-positions; for flip stages b_sl is reversed, so read the
    # scratch reversed against forward b-positions instead.
    st = b_sl.indices(q)[2]
    if st > 0:
        nc.gpsimd.tensor_tensor(dout[:, :, :, b_sl], b, sc, AluOpType.add)
    else:
        fwd = slice(q - w, q)  # forward span of the reversed b_sl
        nc.gpsimd.tensor_tensor(dout[:, :, :, fwd], din[:, :, :, fwd],
                                sc[:, :, :, ::-1], AluOpType.add)


# ---------------------------------------------------------------- device kernel

def object_raysampler_kernel(tc, outs, ins, n_rays=CORE_RAYS, g=G):
    nc = tc.nc
    st_rays = P * g
    n_st = n_rays // st_rays
    stages = build_stages2()
    lower_st, upper_st = build_final_merge()

    org_d, dir_d, len_d = ins["origins"], ins["directions"], ins["lengths"]
    traf_d, rots_d, sdiag_d = ins["traf"], ins["rots"], ins["sdiag"]
    iota_d, nodef_d = ins["iota31"], ins["nodef"]
    emat_d, emat2_d, ident_d = ins["emat"], ins["emat2"], ins["ident"]
    slen_d, snode_d, smask_d = outs["slen"], outs["snode"], outs["smask"]
    pts_d, dirs_d = outs["pts"], outs["dirso"]

    ACTF = mybir.ActivationFunctionType

    with contextlib.ExitStack() as ctx:
        cpool = ctx.enter_context(tc.tile_pool(name="const", bufs=1))
        pool = ctx.enter_context(tc.tile_pool(name="work", bufs=2))
        kpool = ctx.enter_context(tc.tile_pool(name="keys", bufs=2))
        ppool = ctx.enter_context(tc.tile_pool(name="psum", bufs=2, space="PSUM"))

        # ---- constants / transforms (once) ----
        traf_t = cpool.tile([1, 192], F32)
        nc.sync.dma_start(traf_t[:], traf_d[:])
        rots_t = cpool.tile([1, 192], F32)
        nc.sync.dma_start(rots_t[:], rots_d[:])
        sdiag_t = cpool.tile([1, 192], F32)
        nc.sync.dma_start(sdiag_t[:], sdiag_d[:])
        iota_r = cpool.tile([1, 512], F32)
        nc.sync.dma_start(iota_r[:], iota_d[:])
        nodef_r = cpool.tile([1, 16], F32)
        nc.sync.dma_start(nodef_r[:], nodef_d[:])
        emat_t = cpool.tile([96, 1536], F32)
        nc.sync.dma_start(emat_t[:], emat_d[:])
        emat2_t = cpool.tile([48, 1536], F32)
        nc.sync.dma_start(emat2_t[:], emat2_d[:])
        ident_t = cpool.tile([128, 128], F32)
        nc.sync.dma_start(ident_t[:], ident_d[:])

        pt1 = cpool.tile([1, 192], F32)     # trafos @ scales, rows (i, (j,m))
        nc.vector.tensor_mul(pt1[:], traf_t[:], sdiag_t[:])
        dt1 = cpool.tile([1, 192], F32)     # rots @ scales
        nc.vector.tensor_mul(dt1[:], rots_t[:], sdiag_t[:])
        ptb = []
        for i in range(4):
            t = cpool.tile([P, 48], F32, tag=f"ptb{i}")
            nc.gpsimd.partition_broadcast(t[:], pt1[:, i * 48:(i + 1) * 48])
            ptb.append(t)
        dtb = []
        for i in range(3):
            t = cpool.tile([P, 48], F32, tag=f"dtb{i}")
            nc.gpsimd.partition_broadcast(t[:], dt1[:, i * 48:(i + 1) * 48])
            dtb.append(t)
        iota_b = cpool.tile([P, 512], F32)
        nc.gpsimd.partition_broadcast(iota_b[:], iota_r[:])
        nodef_b = cpool.tile([P, 16], F32)
        nc.gpsimd.partition_broadcast(nodef_b[:], nodef_r[:])
        c1e10 = cpool.tile([P, g * 16], F32)
        nc.vector.memset(c1e10[:], MISS)
        czero = cpool.tile([P, g * 16], F32)
        nc.vector.memset(czero[:], 0.0)

        iota_g = iota_b[:].rearrange("p (m s) -> p m s", m=M).unsqueeze(1) \
            .broadcast_to([P, g, M, S])

        TT = nc.vector.tensor_tensor
        TS = nc.vector.tensor_scalar
        CP = nc.vector.tensor_copy
        gT = nc.gpsimd.tensor_tensor
        gC = nc.gpsimd.tensor_copy
        ACT = nc.scalar.activation

        def b3(x):
            # [P, g, 16] view -> [P, g, 3, 16] broadcast over j
            return x.unsqueeze(2).broadcast_to([P, g, 3, 16])

        def bj(t48):
            return t48[:].rearrange("p (g j m) -> p g j m", g=g, j=3)

        def frontend(t):
            r0 = t * st_rays
            org = pool.tile([P, g * 3], F32)
            nc.sync.dma_start(org[:].rearrange("p (g j) -> p g j", g=g),
                              org_d[r0:r0 + st_rays, :].rearrange("(g p) j -> p g j", g=g))
            dirw = pool.tile([P, g * 3], F32)
            nc.sync.dma_start(dirw[:].rearrange("p (g j) -> p g j", g=g),
                              dir_d[r0:r0 + st_rays, :].rearrange("(g p) j -> p g j", g=g))

            orgv = org[:].rearrange("p (g j) -> p g j", g=g)
            dirv = dirw[:].rearrange("p (g j) -> p g j", g=g)

            # ||d|| -> nrm [P, g]
            d2 = pool.tile([P, g * 3], F32)
            ACT(d2[:], dirw[:], ACTF.Square)
            d2v = d2[:].rearrange("p (g j) -> p g j", g=g)
            nd = pool.tile([P, g], F32)
            ndv = nd[:].unsqueeze(2)
            TT(ndv, d2v[:, :, 0:1], d2v[:, :, 1:2], AluOpType.add)
            TT(ndv, ndv, d2v[:, :, 2:3], AluOpType.add)
            nrm = pool.tile([P, g], F32)
            ACT(nrm[:], nd[:], ACTF.Sqrt)

            # origins/dirs in object frames: layout (g, j, m)
            oo = pool.tile([P, g * 48], F32)
            oov = bj(oo)
            tmp48 = pool.tile([P, g * 48], F32)
            tmpv = bj(tmp48)
            pb = [x[:].unsqueeze(1).broadcast_to([P, g, 48])
                  .rearrange("p g (j m) -> p g j m", j=3) for x in ptb]
            db = [x[:].unsqueeze(1).broadcast_to([P, g, 48])
                  .rearrange("p g (j m) -> p g j m", j=3) for x in dtb]

            def colb(v, i):
                # [P, g, 3] column i -> [P, g, 3, 16] broadcast
                return v[:, :, i:i + 1].unsqueeze(3).broadcast_to([P, g, 1, 16]) \
                    .broadcast_to([P, g, 3, 16])

            gT(oov, pb[0], colb(orgv, 0), AluOpType.mult)
            gT(tmpv, pb[1], colb(orgv, 1), AluOpType.mult)
            gT(oov, oov, tmpv, AluOpType.add)
            gT(tmpv, pb[2], colb(orgv, 2), AluOpType.mult)
            gT(oov, oov, tmpv, AluOpType.add)
            gT(oov, oov, pb[3], AluOpType.add)

            uu = pool.tile([P, g * 48], F32)
            uuv = bj(uu)
            gT(uuv, db[0], colb(dirv, 0), AluOpType.mult)
            gT(tmpv, db[1], colb(dirv, 1), AluOpType.mult)
            gT(uuv, uuv, tmpv, AluOpType.add)
            gT(tmpv, db[2], colb(dirv, 2), AluOpType.mult)
            gT(uuv, uuv, tmpv, AluOpType.add)

            u2 = pool.tile([P, g * 48], F32)
            ACT(u2[:], uu[:], ACTF.Square)
            u2v = bj(u2)
            n2 = pool.tile([P, g * 16], F32)
            n2v = n2[:].rearrange("p (g m) -> p g m", g=g)
            gT(n2v, u2v[:, :, 0], u2v[:, :, 1], AluOpType.add)
            gT(n2v, n2v, u2v[:, :, 2], AluOpType.add)
            no = pool.tile([P, g * 16], F32)
            ACT(no[:], n2[:], ACTF.Sqrt)
            rno = pool.tile([P, g * 16], F32)
            nc.vector.reciprocal(rno[:], no[:])
            rnov = rno[:].rearrange("p (g m) -> p g m", g=g)

            cc = pool.tile([P, g * 16], F32)
            ccv = cc[:].rearrange("p (g m) -> p g m", g=g)
            nrm_b = nrm[:].unsqueeze(2).broadcast_to([P, g, 16])
            gT(ccv, rnov, nrm_b, AluOpType.mult)

            un = pool.tile([P, g * 48], F32)
            unv = bj(un)
            gT(unv, uuv, b3(rnov), AluOpType.mult)
            inv = pool.tile([P, g * 48], F32)
            nc.vector.reciprocal(inv[:], un[:])

            w0 = pool.tile([P, g * 48], F32)
            ACT(w0[:], oo[:], ACTF.Copy, bias=-1.0, scale=-1.0)
            t0 = pool.tile([P, g * 48], F32)
            gT(t0[:], w0[:], inv[:], AluOpType.mult)
            w1 = pool.tile([P, g * 48], F32)
            ACT(w1[:], oo[:], ACTF.Copy, bias=1.0, scale=-1.0)
            t1 = pool.tile([P, g * 48], F32)
            gT(t1[:], w1[:], inv[:], AluOpType.mult)

            lo = pool.tile([P, g * 48], F32)
            TT(lo[:], t0[:], t1[:], AluOpType.min)
            hi = pool.tile([P, g * 48], F32)
            TT(hi[:], t0[:], t1[:], AluOpType.max)
            lov, hiv = bj(lo), bj(hi)

            tmn = pool.tile([P, g * 16], F32)
            tmnv = tmn[:].rearrange("p (g m) -> p g m", g=g)
            TT(tmnv, lov[:, :, 0], lov[:, :, 1], AluOpType.max)
            TT(tmnv, tmnv, lov[:, :, 2], AluOpType.max)
            tmx = pool.tile([P, g * 16], F32)
            tmxv = tmx[:].rearrange("p (g m) -> p g m", g=g)
            TT(tmxv, hiv[:, :, 0], hiv[:, :, 1], AluOpType.min)
            TT(tmxv, tmxv, hiv[:, :, 2], AluOpType.min)

            hit = pool.tile([P, g * 16], F32)
            TT(hit[:], tmx[:], tmn[:], AluOpType.is_gt)
            h2 = pool.tile([P, g * 16], F32)
            TS(h2[:], tmx[:], 0.0, None, AluOpType.is_gt)
            gT(hit[:], hit[:], h2[:], AluOpType.mult)
            hitv = hit[:].rearrange("p (g m) -> p g m", g=g)

            tin = pool.tile([P, g * 16], F32)
            ACT(tin[:], tmn[:], ACTF.Relu)
            wdt = pool.tile([P, g * 16], F32)
            gT(wdt[:], tmx[:], tin[:], AluOpType.subtract)
            tinv = tin[:].rearrange("p (g m) -> p g m", g=g)

            a0 = pool.tile([P, g * 16], F32)
            gT(a0[:], cc[:], tin[:], AluOpType.mult)
            b0 = pool.tile([P, g * 16], F32)
            gT(b0[:], cc[:], wdt[:], AluOpType.mult)

            am = pool.tile([P, g * 16], F32)
            CP(am[:], c1e10[:])
            nc.vector.copy_predicated(am[:], hit[:].bitcast(I32), a0[:])
            bm = pool.tile([P, g * 16], F32)
            CP(bm[:], czero[:])
            nc.vector.copy_predicated(bm[:], hit[:].bitcast(I32), b0[:])

            nbf = pool.tile([P, g * 16], F32)
            gT(nbf[:].rearrange("p (g m) -> p g m", g=g), hitv,
               nodef_b[:].unsqueeze(1).broadcast_to([P, g, 16]), AluOpType.mult)
            nbi = pool.tile([P, g * 16], I32)
            CP(nbi[:], nbf[:])

            # ---- keys ----
            ka = kpool.tile([P, g * K], F32)
            kb = kpool.tile([P, g * K], F32)
            kav4 = ka[:].rearrange("p (g k) -> p g k", g=g)[:, :, 0:512] \
                .rearrange("p g (m s) -> p g m s", m=M)
            amb = am[:].rearrange("p (g m) -> p g m", g=g).unsqueeze(3) \
                .broadcast_to([P, g, M, S])
            bmb = bm[:].rearrange("p (g m) -> p g m", g=g).unsqueeze(3) \
                .broadcast_to([P, g, M, S])
            gT(kav4, bmb, iota_g, AluOpType.mult)
            gT(kav4, kav4, amb, AluOpType.add)
            kai = ka[:].bitcast(I32).rearrange("p (g k) -> p g k", g=g)
            nc.sync.dma_start(ka[:].rearrange("p (g k) -> p g k", g=g)[:, :, 512:576],
                              len_d[r0:r0 + st_rays, :].rearrange("(g p) k -> p g k", g=g))
            TS(kai[:, :, 0:K], kai[:, :, 0:K], -32, None, AluOpType.bitwise_and)
            nbib = nbi[:].rearrange("p (g m) -> p g m", g=g).unsqueeze(3) \
                .broadcast_to([P, g, M, S])
            kai4 = kai[:, :, 0:512].rearrange("p g (m s) -> p g m s", m=M)
            TT(kai4, kai4, nbib, AluOpType.bitwise_or)

            # ---- sample points / dirs via TensorE: [P|Q] @ E, [D] @ E2 ----
            wdt31 = pool.tile([P, g * 16], F32)
            TS(wdt31[:], wdt[:], float(1.0 / 31.0), None, AluOpType.mult)
            w31v = wdt31[:].rearrange("p (g m) -> p g m", g=g)

            pqd = pool.tile([P, g * 144], F32)
            pqdv = pqd[:].rearrange("p (g c) -> p g c", g=g)
            pv = pqdv[:, :, 0:48].rearrange("p g (j m) -> p g j m", j=3)
            qv = pqdv[:, :, 48:96].rearrange("p g (j m) -> p g j m", j=3)
            dvv = pqdv[:, :, 96:144].rearrange("p g (j m) -> p g j m", j=3)
            gT(pv, unv, b3(tinv), AluOpType.mult)
            gT(pv, pv, oov, AluOpType.add)
            gT(pv, pv, b3(hitv), AluOpType.mult)
            gT(qv, unv, b3(w31v), AluOpType.mult)
            gT(qv, qv, b3(hitv), AluOpType.mult)
            gT(dvv, unv, b3(hitv), AluOpType.mult)

            for gg in range(g):
                pq_ps = ppool.tile([96, 128], F32, tag="pqps")
                nc.tensor.transpose(pq_ps[:], pqdv[:, gg, 0:96], ident_t[:])
                pqT = pool.tile([96, 128], F32, tag="pqT")
                ACT(pqT[:], pq_ps[:], ACTF.Copy)
                d_ps = ppool.tile([48, 128], F32, tag="dps")
                nc.tensor.transpose(d_ps[:], pqdv[:, gg, 96:144], ident_t[:])
                dT = pool.tile([48, 128], F32, tag="dT")
                ACT(dT[:], d_ps[:], ACTF.Copy)

                pts_t = pool.tile([P, 1536], F32, tag="ptst")
                dirs_t = pool.tile([P, 1536], F32, tag="dirst")
                for c in range(3):
                    mm = ppool.tile([128, 512], F32, tag="mmps")
                    nc.tensor.matmul(mm[:], pqT[:], emat_t[:, 512 * c:512 * (c + 1)])
                    ACT(pts_t[:, 512 * c:512 * (c + 1)], mm[:], ACTF.Copy)
                    mm2 = ppool.tile([128, 512], F32, tag="mmps2")
                    nc.tensor.matmul(mm2[:], dT[:], emat2_t[:, 512 * c:512 * (c + 1)])
                    ACT(dirs_t[:, 512 * c:512 * (c + 1)], mm2[:], ACTF.Copy)

                rg = r0 + gg * P
                nc.sync.dma_start(pts_d[:, rg:rg + P, :].transpose([1, 0, 2]),
                                  pts_t[:].rearrange("p (m f) -> p m f", m=M))
                nc.sync.dma_start(dirs_d[:, rg:rg + P, :].transpose([1, 0, 2]),
                                  dirs_t[:].rearrange("p (m f) -> p m f", m=M))

            return dict(ka=ka, kb=kb, r0=r0)

        def sort1(stt):
            # leading stages on GpSimd (exact int32 compare-exchange)
            ka, kb = stt["ka"], stt["kb"]
            scratch = pool.tile([P, g * 256], I32, tag="posc")
            sb, db = ka, kb
            for stage in stages[:N_POOL_SORT]:
                _emit_pool_stage(nc, db, sb, stage, g, scratch[:])
                sb, db = db, sb

        def backend(stt):
            ka, kb, r0 = stt["ka"], stt["kb"], stt["r0"]
            # remaining A-D stages on DVE (N_POOL_SORT is even -> src is ka)
            sb, db = ka, kb
            for stage in stages[N_POOL_SORT:]:
                _emit_stage(nc, "v", db[:], sb[:], stage, g)
                sb, db = db, sb
            assert sb is ka
            # compact final-merge flip, in-place on ka:
            # stash original base, then max (reads originals), then min.
            tmp64 = pool.tile([P, g * 64], F32, tag="tmp64")
            kav = ka[:].rearrange("p (g k) -> p g k", g=g)
            CP(tmp64[:].rearrange("p (g k) -> p g k", g=g), kav[:, :, 512:576])
            _emit_stage(nc, "v", ka[:], ka[:],
                        [("max", (512, 1, 64), (511, -1, 64), (512, 1, 64))], g)
            nc.vector.tensor_tensor(
                kav[:, :, 448:512], kav[:, :, 448:512],
                tmp64[:].rearrange("p (g k) -> p g k", g=g)[:, :, 63::-1],
                AluOpType.min)
            # lower [0,512): 9 stages ka -> ... -> kb
            sb, db = ka, kb
            for stage in lower_st:
                _emit_stage(nc, "v", db[:], sb[:], stage, g)
                sb, db = db, sb
            assert sb is kb
            # upper [512,576): 6 stages ka -> ... -> ka, then 1 copy into kb
            sb, db = ka, kb
            for stage in upper_st:
                _emit_stage(nc, "v", db[:], sb[:], stage, g)
                sb, db = db, sb
            assert sb is ka
            _emit_stage(nc, "v", kb[:], ka[:],
                        [("copy", (512, 1, 64), (512, 1, 64), None)], g)
            skey = kb

            # ---- extraction ----
            it5 = pool.tile([P, g * K], I32)
            TS(it5[:], skey[:].bitcast(I32), 31, None, AluOpType.bitwise_and)
            node_t = pool.tile([P, g * K], I32)
            ACT(node_t[:], it5[:], ACTF.Copy, bias=-1.0)
            mask_t = pool.tile([P, g * K], U8)
            ACT(mask_t[:], it5[:], ACTF.Sign)

            nc.scalar.dma_start(slen_d[r0:r0 + st_rays, :].rearrange("(g p) k -> p g k", g=g),
                                skey[:].rearrange("p (g k) -> p g k", g=g))
            nc.scalar.dma_start(snode_d[r0:r0 + st_rays, :].rearrange("(g p) k -> p g k", g=g),
                                node_t[:].rearrange("p (g k) -> p g k", g=g))
            nc.scalar.dma_start(smask_d[r0:r0 + st_rays, :].rearrange("(g p) k -> p g k", g=g),
                                mask_t[:].rearrange("p (g k) -> p g k", g=g))

        window = []
        for t in range(n_st):
            window.append(frontend(t))
            if len(window) >= 2:
                sort1(window[-2])
            if len(window) >= 3:
                backend(window[-3])
        if len(window) >= 2:
            sort1(window[-1])
            backend(window[-2])
        else:
            sort1(window[-1])
        backend(window[-1])



# ---------------------------------------------------------------- host wrapper

def _make_const_inputs(trafos_w2o, rots_w2o, scales_w2o):
    # rows (i, (j, m)) as [4, 48]
    traf = np.ascontiguousarray(trafos_w2o[:, 0:4, 0:3].transpose(1, 2, 0)).reshape(1, 192)
    rots = np.ascontiguousarray(rots_w2o[:, 0:4, 0:3].transpose(1, 2, 0)).reshape(1, 192)
    sd = np.stack([scales_w2o[:, 0, 0], scales_w2o[:, 1, 1], scales_w2o[:, 2, 2]], axis=0)
    sdiag = np.tile(sd.reshape(1, 48), (1, 4)).reshape(1, 192).astype(np.float32)
    lin = np.linspace(0.0, 1.0, S, dtype=np.float32)
    iota31 = np.tile(lin, M).reshape(1, 512).astype(np.float32)
    nodef = (np.arange(M, dtype=np.float32) + 1.0).reshape(1, 16)

    # selector matrices: col index = m*96 + s*3 + j
    cols = np.arange(1536)
    cm = cols // 96
    cs = (cols % 96) // 3
    cj = cols % 3
    emat = np.zeros((96, 1536), dtype=np.float32)
    emat2 = np.zeros((48, 1536), dtype=np.float32)
    emat[cj * 16 + cm, cols] = 1.0
    emat[48 + cj * 16 + cm, cols] = cs.astype(np.float32)
    emat2[cj * 16 + cm, cols] = 1.0
    ident = np.eye(128, dtype=np.float32)
    return (traf.astype(np.float32), rots.astype(np.float32), sdiag, iota31,
            nodef, emat, emat2, ident)


_COMPILED = {}


def _get_compiled(n_rays=CORE_RAYS, g=G):
    key = (n_rays, g)
    if key in _COMPILED:
        return _COMPILED[key]
    nc = bacc.Bacc("TRN2", target_bir_lowering=False, debug=False,
                   num_devices=N_CORES)
    ins = {
        "origins": nc.dram_tensor("origins", [n_rays, 3], F32, kind="ExternalInput").ap(),
        "directions": nc.dram_tensor("directions", [n_rays, 3], F32, kind="ExternalInput").ap(),
        "lengths": nc.dram_tensor("lengths", [n_rays, B], F32, kind="ExternalInput").ap(),
        "traf": nc.dram_tensor("traf", [1, 192], F32, kind="ExternalInput").ap(),
        "rots": nc.dram_tensor("rots", [1, 192], F32, kind="ExternalInput").ap(),
        "sdiag": nc.dram_tensor("sdiag", [1, 192], F32, kind="ExternalInput").ap(),
        "iota31": nc.dram_tensor("iota31", [1, 512], F32, kind="ExternalInput").ap(),
        "nodef": nc.dram_tensor("nodef", [1, 16], F32, kind="ExternalInput").ap(),
        "emat": nc.dram_tensor("emat", [96, 1536], F32, kind="ExternalInput").ap(),
        "emat2": nc.dram_tensor("emat2", [48, 1536], F32, kind="ExternalInput").ap(),
        "ident": nc.dram_tensor("ident", [128, 128], F32, kind="ExternalInput").ap(),
    }
    outs = {
        "slen": nc.dram_tensor("slen", [n_rays, K], F32, kind="ExternalOutput").ap(),
        "snode": nc.dram_tensor("snode", [n_rays, K], I32, kind="ExternalOutput").ap(),
        "smask": nc.dram_tensor("smask", [n_rays, K], U8, kind="ExternalOutput").ap(),
        "pts": nc.dram_tensor("pts", [M, n_rays, S * 3], F32, kind="ExternalOutput").ap(),
        "dirso": nc.dram_tensor("dirso", [M, n_rays, S * 3], F32, kind="ExternalOutput").ap(),
    }
    with tile.TileContext(nc) as tc:
        object_raysampler_kernel(tc, outs, ins, n_rays=n_rays, g=g)
    nc.compile()
    _COMPILED[key] = nc
    return nc


def kernel(origins, directions, lengths, trafos_w2o, rots_w2o, scales_w2o,
           _trace=False, _trace_kwargs=None):
    origins = np.asarray(origins, dtype=np.float32)
    directions = np.asarray(directions, dtype=np.float32)
    lengths = np.asarray(lengths, dtype=np.float32)
    traf, rots, sdiag, iota31, nodef, emat, emat2, ident = _make_const_inputs(
        np.asarray(trafos_w2o, np.float32), np.asarray(rots_w2o, np.float32),
        np.asarray(scales_w2o, np.float32))

    nc = _get_compiled()
    in_maps = []
    for c in range(N_CORES):
        r0 = c * CORE_RAYS
        in_maps.append({
            "origins": origins[r0:r0 + CORE_RAYS],
            "directions": directions[r0:r0 + CORE_RAYS],
            "lengths": lengths[r0:r0 + CORE_RAYS],
            "traf": traf, "rots": rots, "sdiag": sdiag,
            "iota31": iota31, "nodef": nodef,
            "emat": emat, "emat2": emat2, "ident": ident,
        })
    kwargs = {}
    if _trace:
        kwargs = dict(trace=True, **(_trace_kwargs or {}))
    res = run_bass_kernel_spmd(nc, in_maps, list(range(N_CORES)), **kwargs)
    results = res.results

    slen = np.concatenate([results[c]["slen"] for c in range(N_CORES)], axis=0)
    snode = np.concatenate([results[c]["snode"] for c in range(N_CORES)], axis=0)
    smask = np.concatenate([results[c]["smask"] for c in range(N_CORES)], axis=0)
    pts = np.concatenate([results[c]["pts"] for c in range(N_CORES)], axis=1)
    dirso = np.concatenate([results[c]["dirso"] for c in range(N_CORES)], axis=1)
    pts_flat = pts.reshape(-1, 3)
    dirs_flat = dirso.reshape(-1, 3)
    out = (slen, snode.astype(np.int32), smask.astype(bool), pts_flat, dirs_flat)
    if _trace:
        return out, res
    return out


# revision 18
# speedup vs baseline: 16825.5182x; 16825.5182x over previous
"""Trainium2 Bass kernel for nn_ObjectRaysampler.

Full (unsharded) inputs -> full outputs. Rays are sharded across 8 NeuronCores
(data-parallel); the tiny object transforms are replicated.

Per super-tile of G*128 rays (128 partitions x G ray-groups in the free dim):
  - world->object transform of ray origins/directions (row-vector convention),
    slab ray-AABB test against the unit box, entry/exit t.
  - world-space sample depths are z_w = c * z_o with c = ||d|| / ||d @ (R S)||,
    so each object's 32 samples form an ascending arithmetic progression:
    z(s) = A + B*(s/31), A = c*t_in, B = c*(tmax-t_in); misses get exactly 1e10.
  - sort keys: the f32 depth with its low 5 mantissa bits replaced by
    (node_id+1) for hits and 0 for misses/base entries. Keys stay positive
    floats, so min/max comparisons reproduce the reference's stable argsort
    order (remaining ties are between interchangeable entries), and node id /
    hit mask are recovered from the sorted keys by bit masking. The <=31-ulp
    key perturbation is ~4e-6 relative on the output lengths.
  - the per-ray 576 values are 17 presorted runs (16 arithmetic progressions
    + the presorted base lengths), merged with a bitonic-merge network of
    40 stages / ~96 min-max ops on 576-slot ping-pong buffers (the inf-pad
    of the final merge provably collapses to 64-wide ops). The network runs
    on VectorE (the only engine with exact min/max); emission is software-
    pipelined so the frontend of super-tile t (GpSimd/ScalarE/TensorE/DMA)
    overlaps the sort of super-tile t-1.
  - sample points/dirs are affine in s: pts = P + Q*s per (ray, object, xyz),
    so TensorE produces them as [P|Q] @ E against a constant selector matrix,
    ScalarE drains PSUM->SBUF, and DMA streams them to HBM.
"""

import contextlib
import numpy as np

from concourse import bacc, tile, mybir
from concourse.alu_op_type import AluOpType
from concourse.bass_utils import run_bass_kernel_spmd

N_RAYS = 32768
M = 16          # objects
S = 32          # samples per object
B = 64          # base samples
K = B + M * S   # 576
MISS = 1e10
N_CORES = 8
P = 128                        # partition dim (rays per group)
G = 4                          # ray-groups per super-tile
CORE_RAYS = N_RAYS // N_CORES  # 4096
F32 = mybir.dt.float32
I32 = mybir.dt.int32
U8 = mybir.dt.uint8

# ---------------------------------------------------------------- sort network


def _flip(L, lo, hi):
    nblk = (hi - lo) // (2 * L)
    q = 2 * L
    return [
        ("min", lo, nblk, q, slice(0, L), slice(0, L), slice(2 * L - 1, L - 1, -1)),
        ("max", lo, nblk, q, slice(L, 2 * L), slice(L, 2 * L), slice(L - 1, None, -1)),
    ]


def _plain(d, lo, hi):
    nblk = (hi - lo) // (2 * d)
    q = 2 * d
    return [
        ("min", lo, nblk, q, slice(0, d), slice(0, d), slice(d, 2 * d)),
        ("max", lo, nblk, q, slice(d, 2 * d), slice(0, d), slice(d, 2 * d)),
    ]


N_POOL_SORT = 0   # GpSimd sort assist disabled: POOL ALU is float-only,
                  # so exact int32 compare-exchange is impossible there


def build_stages2():
    stages = []
    for L in (32, 64, 128, 256):            # merge 32-runs -> one 512-run in [0,512)
        stages.append(_flip(L, 0, 512))
        d = L // 2
        while d >= 1:
            stages.append(_plain(d, 0, 512))
            d //= 2
    return stages


def build_final_merge():
    """Returns (lower_stages, upper_stages) for the compact final merge.

    The virtual-inf-pad flip of the 1024-wide bitonic merge collapses to the
    64 real pairs and runs IN-PLACE (max first, reading only originals; min
    then reads the stashed original base from a temp). lower_stages sort the
    bitonic [0,512); upper_stages sort the bitonic [512,576) independently.
    """
    lower = [_plain(d, 0, 512) for d in (256, 128, 64, 32, 16, 8, 4, 2, 1)]
    upper = [_plain(d, 512, 576) for d in (32, 16, 8, 4, 2, 1)]
    return lower, upper


_ALU = {"min": AluOpType.min, "max": AluOpType.max}


def _spec_ap(buf, g, spec):
    """buf: [P, G*K] AP; spec = (off, step, cnt) within each group's K slots."""
    off, step, cnt = spec
    v = buf.rearrange("p (g k) -> p g k", g=g)
    if step > 0:
        return v[:, :, off:off + cnt]
    stop = off - cnt
    return v[:, :, off:stop if stop >= 0 else None:-1]


def _emit_stage(nc, eng, dst, src, stage, g):
    e = nc.vector if eng == "v" else nc.gpsimd
    for op in stage:
        kind = op[0]
        if isinstance(op[1], tuple):
            _, o, a, b = op
            dv = _spec_ap(dst, g, o)
            av = _spec_ap(src, g, a)
            bv = _spec_ap(src, g, b) if b is not None else None
        else:
            _, off, nblk, q, o_sl, a_sl, b_sl = op
            dvv = dst.rearrange("p (g k) -> p g k", g=g)[:, :, off:off + nblk * q]
            avv = src.rearrange("p (g k) -> p g k", g=g)[:, :, off:off + nblk * q]
            dv = dvv.rearrange("p g (b q) -> p g b q", b=nblk)[:, :, :, o_sl]
            av = avv.rearrange("p g (b q) -> p g b q", b=nblk)[:, :, :, a_sl]
            bv = avv.rearrange("p g (b q) -> p g b q", b=nblk)[:, :, :, b_sl]
        if kind == "copy":
            e.tensor_copy(dv, av)
        else:
            e.tensor_tensor(dv, av, bv, _ALU[kind])


def _pair_views(dst, src, g, op_min):
    """From a 'min' op tuple, build (in0, in1, out_min, out_max_in_fwd, r_rev?)
    views: in0/in1 are the comparator sides; outputs at in0/in1 positions."""
    if isinstance(op_min[1], tuple):
        _, o, a, b = op_min
        raise NotImplementedError
    _, off, nblk, q, o_sl, a_sl, b_sl = op_min
    dvv = dst.rearrange("p (g k) -> p g k", g=g)[:, :, off:off + nblk * q]
    svv = src.rearrange("p (g k) -> p g k", g=g)[:, :, off:off + nblk * q]
    din = svv.rearrange("p g (b q) -> p g b q", b=nblk)
    dout = dvv.rearrange("p g (b q) -> p g b q", b=nblk)
    return din, dout, a_sl, b_sl


def _emit_pool_stage(nc, dst, src, stage, g, scratch):
    """Exact compare-exchange on GpSimd via int32 arithmetic:
    r = relu(a - b); min = a - r at a-positions; max = b + r at b-positions.
    Works because positive-f32 key order == int32 order and int ops are exact.
    `stage` must be a [min, max] pair from _flip/_plain."""
    I32 = mybir.dt.int32
    op_min, op_max = stage
    assert op_min[0] == "min" and op_max[0] == "max"
    _, off, nblk, q, o_sl, a_sl, b_sl = op_min
    din, dout, _, _ = _pair_views(dst[:].bitcast(I32), src[:].bitcast(I32), g, op_min)
    a = din[:, :, :, a_sl]
    b = din[:, :, :, b_sl]
    w = a.shape[-1]
    sc = scratch[:, 0:nblk * g * w * 0 + g * nblk * w].rearrange(
        "p (g b w) -> p g b w", g=g, b=nblk)
    nc.gpsimd.tensor_tensor(sc, a, b, AluOpType.subtract)
    nc.gpsimd.tensor_relu(sc, sc)
    nc.gpsimd.tensor_tensor(dout[:, :, :, a_sl], a, sc, AluOpType.subtract)
    # max at b-positions; for flip stages b_sl is reversed, so read the
    # scratch reversed against forward b-positions instead.
    st = b_sl.indices(q)[2]
    if st > 0:
        nc.gpsimd.tensor_tensor(dout[:, :, :, b_sl], b, sc, AluOpType.add)
    else:
        fwd = slice(q - w, q)  # forward span of the reversed b_sl
        nc.gpsimd.tensor_tensor(dout[:, :, :, fwd], din[:, :, :, fwd],
                                sc[:, :, :, ::-1], AluOpType.add)


# ---------------------------------------------------------------- device kernel

def object_raysampler_kernel(tc, outs, ins, n_rays=CORE_RAYS, g=G):
    nc = tc.nc
    st_rays = P * g
    n_st = n_rays // st_rays
    stages = build_stages2()
    lower_st, upper_st = build_final_merge()

    org_d, dir_d, len_d = ins["origins"], ins["directions"], ins["lengths"]
    traf_d, rots_d, sdiag_d = ins["traf"], ins["rots"], ins["sdiag"]
    iota_d, nodef_d = ins["iota31"], ins["nodef"]
    emat_d, emat2_d, ident_d = ins["emat"], ins["emat2"], ins["ident"]
    slen_d, snode_d, smask_d = outs["slen"], outs["snode"], outs["smask"]
    pts_d, dirs_d = outs["pts"], outs["dirso"]

    ACTF = mybir.ActivationFunctionType

    with contextlib.ExitStack() as ctx:
        cpool = ctx.enter_context(tc.tile_pool(name="const", bufs=1))
        pool = ctx.enter_context(tc.tile_pool(name="work", bufs=2))
        kpool = ctx.enter_context(tc.tile_pool(name="keys", bufs=2))
        ppool = ctx.enter_context(tc.tile_pool(name="psum", bufs=2, space="PSUM"))

        # ---- constants / transforms (once) ----
        traf_t = cpool.tile([1, 192], F32)
        nc.sync.dma_start(traf_t[:], traf_d[:])
        rots_t = cpool.tile([1, 192], F32)
        nc.sync.dma_start(rots_t[:], rots_d[:])
        sdiag_t = cpool.tile([1, 192], F32)
        nc.sync.dma_start(sdiag_t[:], sdiag_d[:])
        iota_r = cpool.tile([1, 512], F32)
        nc.sync.dma_start(iota_r[:], iota_d[:])
        nodef_r = cpool.tile([1, 16], F32)
        nc.sync.dma_start(nodef_r[:], nodef_d[:])
        emat_t = cpool.tile([96, 1536], F32)
        nc.sync.dma_start(emat_t[:], emat_d[:])
        emat2_t = cpool.tile([48, 1536], F32)
        nc.sync.dma_start(emat2_t[:], emat2_d[:])
        ident_t = cpool.tile([128, 128], F32)
        nc.sync.dma_start(ident_t[:], ident_d[:])

        pt1 = cpool.tile([1, 192], F32)     # trafos @ scales, rows (i, (j,m))
        nc.vector.tensor_mul(pt1[:], traf_t[:], sdiag_t[:])
        dt1 = cpool.tile([1, 192], F32)     # rots @ scales
        nc.vector.tensor_mul(dt1[:], rots_t[:], sdiag_t[:])
        ptb = []
        for i in range(4):
            t = cpool.tile([P, 48], F32, tag=f"ptb{i}")
            nc.gpsimd.partition_broadcast(t[:], pt1[:, i * 48:(i + 1) * 48])
            ptb.append(t)
        dtb = []
        for i in range(3):
            t = cpool.tile([P, 48], F32, tag=f"dtb{i}")
            nc.gpsimd.partition_broadcast(t[:], dt1[:, i * 48:(i + 1) * 48])
            dtb.append(t)
        iota_b = cpool.tile([P, 512], F32)
        nc.gpsimd.partition_broadcast(iota_b[:], iota_r[:])
        nodef_b = cpool.tile([P, 16], F32)
        nc.gpsimd.partition_broadcast(nodef_b[:], nodef_r[:])
        c1e10 = cpool.tile([P, g * 16], F32)
        nc.vector.memset(c1e10[:], MISS)
        czero = cpool.tile([P, g * 16], F32)
        nc.vector.memset(czero[:], 0.0)

        iota_g = iota_b[:].rearrange("p (m s) -> p m s", m=M).unsqueeze(1) \
            .broadcast_to([P, g, M, S])

        TT = nc.vector.tensor_tensor
        TS = nc.vector.tensor_scalar
        CP = nc.vector.tensor_copy
        gT = nc.gpsimd.tensor_tensor
        gC = nc.gpsimd.tensor_copy
        ACT = nc.scalar.activation

        def b3(x):
            # [P, g, 16] view -> [P, g, 3, 16] broadcast over j
            return x.unsqueeze(2).broadcast_to([P, g, 3, 16])

        def bj(t48):
            return t48[:].rearrange("p (g j m) -> p g j m", g=g, j=3)

        def frontend(t):
            r0 = t * st_rays
            org = pool.tile([P, g * 3], F32)
            nc.sync.dma_start(org[:].rearrange("p (g j) -> p g j", g=g),
                              org_d[r0:r0 + st_rays, :].rearrange("(g p) j -> p g j", g=g))
            dirw = pool.tile([P, g * 3], F32)
            nc.sync.dma_start(dirw[:].rearrange("p (g j) -> p g j", g=g),
                              dir_d[r0:r0 + st_rays, :].rearrange("(g p) j -> p g j", g=g))

            orgv = org[:].rearrange("p (g j) -> p g j", g=g)
            dirv = dirw[:].rearrange("p (g j) -> p g j", g=g)

            # ||d|| -> nrm [P, g]
            d2 = pool.tile([P, g * 3], F32)
            ACT(d2[:], dirw[:], ACTF.Square)
            d2v = d2[:].rearrange("p (g j) -> p g j", g=g)
            nd = pool.tile([P, g], F32)
            ndv = nd[:].unsqueeze(2)
            TT(ndv, d2v[:, :, 0:1], d2v[:, :, 1:2], AluOpType.add)
            TT(ndv, ndv, d2v[:, :, 2:3], AluOpType.add)
            nrm = pool.tile([P, g], F32)
            ACT(nrm[:], nd[:], ACTF.Sqrt)

            # origins/dirs in object frames: layout (g, j, m)
            oo = pool.tile([P, g * 48], F32)
            oov = bj(oo)
            tmp48 = pool.tile([P, g * 48], F32)
            tmpv = bj(tmp48)
            pb = [x[:].unsqueeze(1).broadcast_to([P, g, 48])
                  .rearrange("p g (j m) -> p g j m", j=3) for x in ptb]
            db = [x[:].unsqueeze(1).broadcast_to([P, g, 48])
                  .rearrange("p g (j m) -> p g j m", j=3) for x in dtb]

            def colb(v, i):
                # [P, g, 3] column i -> [P, g, 3, 16] broadcast
                return v[:, :, i:i + 1].unsqueeze(3).broadcast_to([P, g, 1, 16]) \
                    .broadcast_to([P, g, 3, 16])

            gT(oov, pb[0], colb(orgv, 0), AluOpType.mult)
            gT(tmpv, pb[1], colb(orgv, 1), AluOpType.mult)
            gT(oov, oov, tmpv, AluOpType.add)
            gT(tmpv, pb[2], colb(orgv, 2), AluOpType.mult)
            gT(oov, oov, tmpv, AluOpType.add)
            gT(oov, oov, pb[3], AluOpType.add)

            uu = pool.tile([P, g * 48], F32)
            uuv = bj(uu)
            gT(uuv, db[0], colb(dirv, 0), AluOpType.mult)
            gT(tmpv, db[1], colb(dirv, 1), AluOpType.mult)
            gT(uuv, uuv, tmpv, AluOpType.add)
            gT(tmpv, db[2], colb(dirv, 2), AluOpType.mult)
            gT(uuv, uuv, tmpv, AluOpType.add)

            u2 = pool.tile([P, g * 48], F32)
            ACT(u2[:], uu[:], ACTF.Square)
            u2v = bj(u2)
            n2 = pool.tile([P, g * 16], F32)
            n2v = n2[:].rearrange("p (g m) -> p g m", g=g)
            gT(n2v, u2v[:, :, 0], u2v[:, :, 1], AluOpType.add)
            gT(n2v, n2v, u2v[:, :, 2], AluOpType.add)
            no = pool.tile([P, g * 16], F32)
            ACT(no[:], n2[:], ACTF.Sqrt)
            rno = pool.tile([P, g * 16], F32)
            rsc = pool.tile([P, g * 48], F32)
            nc.vector.reciprocal_approx_accurate(rno[:], no[:], rsc[:, 0:g * 16])
            rnov = rno[:].rearrange("p (g m) -> p g m", g=g)

            cc = pool.tile([P, g * 16], F32)
            ccv = cc[:].rearrange("p (g m) -> p g m", g=g)
            nrm_b = nrm[:].unsqueeze(2).broadcast_to([P, g, 16])
            gT(ccv, rnov, nrm_b, AluOpType.mult)

            un = pool.tile([P, g * 48], F32)
            unv = bj(un)
            gT(unv, uuv, b3(rnov), AluOpType.mult)
            inv = pool.tile([P, g * 48], F32)
            nc.vector.reciprocal_approx_accurate(inv[:], un[:], rsc[:])

            w0 = pool.tile([P, g * 48], F32)
            ACT(w0[:], oo[:], ACTF.Copy, bias=-1.0, scale=-1.0)
            t0 = pool.tile([P, g * 48], F32)
            gT(t0[:], w0[:], inv[:], AluOpType.mult)
            w1 = pool.tile([P, g * 48], F32)
            ACT(w1[:], oo[:], ACTF.Copy, bias=1.0, scale=-1.0)
            t1 = pool.tile([P, g * 48], F32)
            gT(t1[:], w1[:], inv[:], AluOpType.mult)

            lo = pool.tile([P, g * 48], F32)
            TT(lo[:], t0[:], t1[:], AluOpType.min)
            hi = pool.tile([P, g * 48], F32)
            TT(hi[:], t0[:], t1[:], AluOpType.max)
            lov, hiv = bj(lo), bj(hi)

            tmn = pool.tile([P, g * 16], F32)
            tmnv = tmn[:].rearrange("p (g m) -> p g m", g=g)
            TT(tmnv, lov[:, :, 0], lov[:, :, 1], AluOpType.max)
            TT(tmnv, tmnv, lov[:, :, 2], AluOpType.max)
            tmx = pool.tile([P, g * 16], F32)
            tmxv = tmx[:].rearrange("p (g m) -> p g m", g=g)
            TT(tmxv, hiv[:, :, 0], hiv[:, :, 1], AluOpType.min)
            TT(tmxv, tmxv, hiv[:, :, 2], AluOpType.min)

            hit = pool.tile([P, g * 16], F32)
            TT(hit[:], tmx[:], tmn[:], AluOpType.is_gt)
            h2 = pool.tile([P, g * 16], F32)
            TS(h2[:], tmx[:], 0.0, None, AluOpType.is_gt)
            gT(hit[:], hit[:], h2[:], AluOpType.mult)
            hitv = hit[:].rearrange("p (g m) -> p g m", g=g)

            tin = pool.tile([P, g * 16], F32)
            ACT(tin[:], tmn[:], ACTF.Relu)
            wdt = pool.tile([P, g * 16], F32)
            gT(wdt[:], tmx[:], tin[:], AluOpType.subtract)
            tinv = tin[:].rearrange("p (g m) -> p g m", g=g)

            a0 = pool.tile([P, g * 16], F32)
            gT(a0[:], cc[:], tin[:], AluOpType.mult)
            b0 = pool.tile([P, g * 16], F32)
            gT(b0[:], cc[:], wdt[:], AluOpType.mult)

            am = pool.tile([P, g * 16], F32)
            ACT(am[:], c1e10[:], ACTF.Copy)
            nc.vector.copy_predicated(am[:], hit[:].bitcast(I32), a0[:])
            bm = pool.tile([P, g * 16], F32)
            ACT(bm[:], czero[:], ACTF.Copy)
            nc.vector.copy_predicated(bm[:], hit[:].bitcast(I32), b0[:])

            nbf = pool.tile([P, g * 16], F32)
            gT(nbf[:].rearrange("p (g m) -> p g m", g=g), hitv,
               nodef_b[:].unsqueeze(1).broadcast_to([P, g, 16]), AluOpType.mult)
            nbi = pool.tile([P, g * 16], I32)
            CP(nbi[:], nbf[:])

            # ---- keys ----
            ka = kpool.tile([P, g * K], F32)
            kb = kpool.tile([P, g * K], F32)
            kav4 = ka[:].rearrange("p (g k) -> p g k", g=g)[:, :, 0:512] \
                .rearrange("p g (m s) -> p g m s", m=M)
            amb = am[:].rearrange("p (g m) -> p g m", g=g).unsqueeze(3) \
                .broadcast_to([P, g, M, S])
            bmb = bm[:].rearrange("p (g m) -> p g m", g=g).unsqueeze(3) \
                .broadcast_to([P, g, M, S])
            gT(kav4, bmb, iota_g, AluOpType.mult)
            gT(kav4, kav4, amb, AluOpType.add)
            kai = ka[:].bitcast(I32).rearrange("p (g k) -> p g k", g=g)
            nc.sync.dma_start(ka[:].rearrange("p (g k) -> p g k", g=g)[:, :, 512:576],
                              len_d[r0:r0 + st_rays, :].rearrange("(g p) k -> p g k", g=g))
            TS(kai[:, :, 0:K], kai[:, :, 0:K], -32, None, AluOpType.bitwise_and)
            nbib = nbi[:].rearrange("p (g m) -> p g m", g=g).unsqueeze(3) \
                .broadcast_to([P, g, M, S])
            kai4 = kai[:, :, 0:512].rearrange("p g (m s) -> p g m s", m=M)
            TT(kai4, kai4, nbib, AluOpType.bitwise_or)

            # ---- sample points / dirs via TensorE: [P|Q] @ E, [D] @ E2 ----
            wdt31 = pool.tile([P, g * 16], F32)
            TS(wdt31[:], wdt[:], float(1.0 / 31.0), None, AluOpType.mult)
            w31v = wdt31[:].rearrange("p (g m) -> p g m", g=g)

            pqd = pool.tile([P, g * 144], F32)
            pqdv = pqd[:].rearrange("p (g c) -> p g c", g=g)
            pv = pqdv[:, :, 0:48].rearrange("p g (j m) -> p g j m", j=3)
            qv = pqdv[:, :, 48:96].rearrange("p g (j m) -> p g j m", j=3)
            dvv = pqdv[:, :, 96:144].rearrange("p g (j m) -> p g j m", j=3)
            gT(pv, unv, b3(tinv), AluOpType.mult)
            gT(pv, pv, oov, AluOpType.add)
            gT(pv, pv, b3(hitv), AluOpType.mult)
            gT(qv, unv, b3(w31v), AluOpType.mult)
            gT(qv, qv, b3(hitv), AluOpType.mult)
            gT(dvv, unv, b3(hitv), AluOpType.mult)

            for gg in range(g):
                pq_ps = ppool.tile([96, 128], F32, tag="pqps")
                nc.tensor.transpose(pq_ps[:], pqdv[:, gg, 0:96], ident_t[:])
                pqT = pool.tile([96, 128], F32, tag="pqT")
                ACT(pqT[:], pq_ps[:], ACTF.Copy)
                d_ps = ppool.tile([48, 128], F32, tag="dps")
                nc.tensor.transpose(d_ps[:], pqdv[:, gg, 96:144], ident_t[:])
                dT = pool.tile([48, 128], F32, tag="dT")
                ACT(dT[:], d_ps[:], ACTF.Copy)

                pts_t = pool.tile([P, 1536], F32, tag="ptst")
                dirs_t = pool.tile([P, 1536], F32, tag="dirst")
                for c in range(3):
                    mm = ppool.tile([128, 512], F32, tag="mmps")
                    nc.tensor.matmul(mm[:], pqT[:], emat_t[:, 512 * c:512 * (c + 1)])
                    ACT(pts_t[:, 512 * c:512 * (c + 1)], mm[:], ACTF.Copy)
                    mm2 = ppool.tile([128, 512], F32, tag="mmps2")
                    nc.tensor.matmul(mm2[:], dT[:], emat2_t[:, 512 * c:512 * (c + 1)])
                    ACT(dirs_t[:, 512 * c:512 * (c + 1)], mm2[:], ACTF.Copy)

                rg = r0 + gg * P
                nc.sync.dma_start(pts_d[:, rg:rg + P, :].transpose([1, 0, 2]),
                                  pts_t[:].rearrange("p (m f) -> p m f", m=M))
                nc.sync.dma_start(dirs_d[:, rg:rg + P, :].transpose([1, 0, 2]),
                                  dirs_t[:].rearrange("p (m f) -> p m f", m=M))

            return dict(ka=ka, kb=kb, r0=r0)

        def sort1(stt):
            # leading stages on GpSimd (exact int32 compare-exchange)
            ka, kb = stt["ka"], stt["kb"]
            scratch = pool.tile([P, g * 256], I32, tag="posc")
            sb, db = ka, kb
            for stage in stages[:N_POOL_SORT]:
                _emit_pool_stage(nc, db, sb, stage, g, scratch[:])
                sb, db = db, sb

        def backend(stt):
            ka, kb, r0 = stt["ka"], stt["kb"], stt["r0"]
            # remaining A-D stages on DVE (N_POOL_SORT is even -> src is ka)
            sb, db = ka, kb
            for stage in stages[N_POOL_SORT:]:
                _emit_stage(nc, "v", db[:], sb[:], stage, g)
                sb, db = db, sb
            assert sb is ka
            # compact final-merge flip, in-place on ka:
            # stash original base, then max (reads originals), then min.
            tmp64 = pool.tile([P, g * 64], F32, tag="tmp64")
            kav = ka[:].rearrange("p (g k) -> p g k", g=g)
            CP(tmp64[:].rearrange("p (g k) -> p g k", g=g), kav[:, :, 512:576])
            _emit_stage(nc, "v", ka[:], ka[:],
                        [("max", (512, 1, 64), (511, -1, 64), (512, 1, 64))], g)
            nc.vector.tensor_tensor(
                kav[:, :, 448:512], kav[:, :, 448:512],
                tmp64[:].rearrange("p (g k) -> p g k", g=g)[:, :, 63::-1],
                AluOpType.min)
            # lower [0,512): 9 stages ka -> ... -> kb
            sb, db = ka, kb
            for stage in lower_st:
                _emit_stage(nc, "v", db[:], sb[:], stage, g)
                sb, db = db, sb
            assert sb is kb
            # upper [512,576): 6 stages ka -> ... -> ka, then 1 copy into kb
            sb, db = ka, kb
            for stage in upper_st:
                _emit_stage(nc, "v", db[:], sb[:], stage, g)
                sb, db = db, sb
            assert sb is ka
            _emit_stage(nc, "v", kb[:], ka[:],
                        [("copy", (512, 1, 64), (512, 1, 64), None)], g)
            skey = kb

            # ---- extraction ----
            it5 = pool.tile([P, g * K], I32)
            TS(it5[:], skey[:].bitcast(I32), 31, None, AluOpType.bitwise_and)
            node_t = pool.tile([P, g * K], I32)
            ACT(node_t[:], it5[:], ACTF.Copy, bias=-1.0)
            mask_t = pool.tile([P, g * K], U8)
            ACT(mask_t[:], it5[:], ACTF.Sign)

            nc.scalar.dma_start(slen_d[r0:r0 + st_rays, :].rearrange("(g p) k -> p g k", g=g),
                                skey[:].rearrange("p (g k) -> p g k", g=g))
            nc.scalar.dma_start(snode_d[r0:r0 + st_rays, :].rearrange("(g p) k -> p g k", g=g),
                                node_t[:].rearrange("p (g k) -> p g k", g=g))
            nc.scalar.dma_start(smask_d[r0:r0 + st_rays, :].rearrange("(g p) k -> p g k", g=g),
                                mask_t[:].rearrange("p (g k) -> p g k", g=g))

        window = []
        for t in range(n_st):
            window.append(frontend(t))
            if len(window) >= 2:
                sort1(window[-2])
            if len(window) >= 3:
                backend(window[-3])
        if len(window) >= 2:
            sort1(window[-1])
            backend(window[-2])
        else:
            sort1(window[-1])
        backend(window[-1])



# ---------------------------------------------------------------- host wrapper

def _make_const_inputs(trafos_w2o, rots_w2o, scales_w2o):
    # rows (i, (j, m)) as [4, 48]
    traf = np.ascontiguousarray(trafos_w2o[:, 0:4, 0:3].transpose(1, 2, 0)).reshape(1, 192)
    rots = np.ascontiguousarray(rots_w2o[:, 0:4, 0:3].transpose(1, 2, 0)).reshape(1, 192)
    sd = np.stack([scales_w2o[:, 0, 0], scales_w2o[:, 1, 1], scales_w2o[:, 2, 2]], axis=0)
    sdiag = np.tile(sd.reshape(1, 48), (1, 4)).reshape(1, 192).astype(np.float32)
    lin = np.linspace(0.0, 1.0, S, dtype=np.float32)
    iota31 = np.tile(lin, M).reshape(1, 512).astype(np.float32)
    nodef = (np.arange(M, dtype=np.float32) + 1.0).reshape(1, 16)

    # selector matrices: col index = m*96 + s*3 + j
    cols = np.arange(1536)
    cm = cols // 96
    cs = (cols % 96) // 3
    cj = cols % 3
    emat = np.zeros((96, 1536), dtype=np.float32)
    emat2 = np.zeros((48, 1536), dtype=np.float32)
    emat[cj * 16 + cm, cols] = 1.0
    emat[48 + cj * 16 + cm, cols] = cs.astype(np.float32)
    emat2[cj * 16 + cm, cols] = 1.0
    ident = np.eye(128, dtype=np.float32)
    return (traf.astype(np.float32), rots.astype(np.float32), sdiag, iota31,
            nodef, emat, emat2, ident)


_COMPILED = {}


def _get_compiled(n_rays=CORE_RAYS, g=G):
    key = (n_rays, g)
    if key in _COMPILED:
        return _COMPILED[key]
    nc = bacc.Bacc("TRN2", target_bir_lowering=False, debug=False,
                   num_devices=N_CORES)
    ins = {
        "origins": nc.dram_tensor("origins", [n_rays, 3], F32, kind="ExternalInput").ap(),
        "directions": nc.dram_tensor("directions", [n_rays, 3], F32, kind="ExternalInput").ap(),
        "lengths": nc.dram_tensor("lengths", [n_rays, B], F32, kind="ExternalInput").ap(),
        "traf": nc.dram_tensor("traf", [1, 192], F32, kind="ExternalInput").ap(),
        "rots": nc.dram_tensor("rots", [1, 192], F32, kind="ExternalInput").ap(),
        "sdiag": nc.dram_tensor("sdiag", [1, 192], F32, kind="ExternalInput").ap(),
        "iota31": nc.dram_tensor("iota31", [1, 512], F32, kind="ExternalInput").ap(),
        "nodef": nc.dram_tensor("nodef", [1, 16], F32, kind="ExternalInput").ap(),
        "emat": nc.dram_tensor("emat", [96, 1536], F32, kind="ExternalInput").ap(),
        "emat2": nc.dram_tensor("emat2", [48, 1536], F32, kind="ExternalInput").ap(),
        "ident": nc.dram_tensor("ident", [128, 128], F32, kind="ExternalInput").ap(),
    }
    outs = {
        "slen": nc.dram_tensor("slen", [n_rays, K], F32, kind="ExternalOutput").ap(),
        "snode": nc.dram_tensor("snode", [n_rays, K], I32, kind="ExternalOutput").ap(),
        "smask": nc.dram_tensor("smask", [n_rays, K], U8, kind="ExternalOutput").ap(),
        "pts": nc.dram_tensor("pts", [M, n_rays, S * 3], F32, kind="ExternalOutput").ap(),
        "dirso": nc.dram_tensor("dirso", [M, n_rays, S * 3], F32, kind="ExternalOutput").ap(),
    }
    with tile.TileContext(nc) as tc:
        object_raysampler_kernel(tc, outs, ins, n_rays=n_rays, g=g)
    nc.compile()
    _COMPILED[key] = nc
    return nc


def kernel(origins, directions, lengths, trafos_w2o, rots_w2o, scales_w2o,
           _trace=False, _trace_kwargs=None):
    origins = np.asarray(origins, dtype=np.float32)
    directions = np.asarray(directions, dtype=np.float32)
    lengths = np.asarray(lengths, dtype=np.float32)
    traf, rots, sdiag, iota31, nodef, emat, emat2, ident = _make_const_inputs(
        np.asarray(trafos_w2o, np.float32), np.asarray(rots_w2o, np.float32),
        np.asarray(scales_w2o, np.float32))

    nc = _get_compiled()
    in_maps = []
    for c in range(N_CORES):
        r0 = c * CORE_RAYS
        in_maps.append({
            "origins": origins[r0:r0 + CORE_RAYS],
            "directions": directions[r0:r0 + CORE_RAYS],
            "lengths": lengths[r0:r0 + CORE_RAYS],
            "traf": traf, "rots": rots, "sdiag": sdiag,
            "iota31": iota31, "nodef": nodef,
            "emat": emat, "emat2": emat2, "ident": ident,
        })
    kwargs = {}
    if _trace:
        kwargs = dict(trace=True, **(_trace_kwargs or {}))
    res = run_bass_kernel_spmd(nc, in_maps, list(range(N_CORES)), **kwargs)
    results = res.results

    slen = np.concatenate([results[c]["slen"] for c in range(N_CORES)], axis=0)
    snode = np.concatenate([results[c]["snode"] for c in range(N_CORES)], axis=0)
    smask = np.concatenate([results[c]["smask"] for c in range(N_CORES)], axis=0)
    pts = np.concatenate([results[c]["pts"] for c in range(N_CORES)], axis=1)
    dirso = np.concatenate([results[c]["dirso"] for c in range(N_CORES)], axis=1)
    pts_flat = pts.reshape(-1, 3)
    dirs_flat = dirso.reshape(-1, 3)
    out = (slen, snode.astype(np.int32), smask.astype(bool), pts_flat, dirs_flat)
    if _trace:
        return out, res
    return out
